# revision 19
# baseline (speedup 1.0000x reference)
"""Trainium2 Bass kernel for nn_PoseCDE.

Mathematical structure exploited (exact, input-independent):
  The CDE integrates over t in [0.1, 1.0], which lies entirely inside the
  FIRST segment of the rectilinear control path (segment grid spacing is 1,
  bucketize gives idx=0 for all eval times).  Segment 0's derivative is the
  time-advance knot: (ts[b,2]-ts[b,1], 0, ..., 0).  Hence
      f(t, z) = s_b * g(z)[:, :, 0]
  and only column 0 of each C-group of Wout matters:  Wsub = Wout[:, ::C].
  The 512 x 262656 matmul collapses to 512 x 512 (verified exact vs the
  reference for arbitrary inputs).

Device computation per core (data-parallel over batch, 8 rows per core):
  36 sequential 3-layer MLP evals (RK4, 9 steps) + linear regressor head.
  All activations are kept TRANSPOSED ([H on partitions, batch on free]) so
  weights are the PE-stationary operand and no on-chip transposes are needed;
  biases become per-partition operands of scalar.activation.

Configs (env POSECDE_WDT = "bf16" | "f32", default bf16):
  bf16: MLP weights + matmul moving operands in bf16 (PSUM accum fp32,
        state/RK4 math fp32).  FWL halves weight-load time on the PE.
  When s (=ts[:,2]-ts[:,1]) is uniform across the batch it is folded into
  the RK4 step scalars (fewer DVE ops, shorter serial chain).
"""

import os
import numpy as np
import ml_dtypes

import concourse.bass as bass
import concourse.bacc as bacc
import concourse.mybir as mybir
from concourse.tile import TileContext
from concourse.bass_utils import run_bass_kernel_spmd

N_CORES = 8
B = 64
BL = B // N_CORES          # batch rows per core
H = 512
C = H + 1
KC = H // 128              # contraction chunks (4)
NSLOT = 10                 # z0 + 9 RK4 states
F32 = mybir.dt.float32
BF16 = mybir.dt.bfloat16
F16 = mybir.dt.float16


def _build_nc(hvals, wdt, fold_s, zero_bias):
    """wdt: "f32"|"bf16"|"f16"; fold_s: None or uniform-s float;
    zero_bias: True specializes the MLP biases to 0 (merged-chunk ACT)."""
    nc = bacc.Bacc(None)
    WDT = {"f32": F32, "bf16": BF16, "f16": F16}[wdt]
    wdt_lp = wdt != "f32"

    w0 = nc.declare_dram_parameter("w0", [H, H], WDT, isOutput=False)
    w1 = nc.declare_dram_parameter("w1", [H, H], WDT, isOutput=False)
    ws = nc.declare_dram_parameter("ws", [H, H], WDT, isOutput=False)
    wr1 = nc.declare_dram_parameter("wr1", [H, 128], F32, isOutput=False)
    wr2 = nc.declare_dram_parameter("wr2", [128, 6], F32, isOutput=False)
    b0 = nc.declare_dram_parameter("b0", [H], F32, isOutput=False)
    b1 = nc.declare_dram_parameter("b1", [H], F32, isOutput=False)
    bs = nc.declare_dram_parameter("bs", [H], F32, isOutput=False)
    br1 = nc.declare_dram_parameter("br1", [128], F32, isOutput=False)
    br2 = nc.declare_dram_parameter("br2", [6], F32, isOutput=False)
    svec = nc.declare_dram_parameter("svec", [BL], F32, isOutput=False)

    posesT = nc.declare_dram_parameter("posesT", [6, NSLOT, BL], F32, isOutput=True)
    hlastT = nc.declare_dram_parameter("hlastT", [128, KC, BL], F32, isOutput=True)

    relu = mybir.ActivationFunctionType.Relu
    tanh = mybir.ActivationFunctionType.Tanh
    ident = mybir.ActivationFunctionType.Identity
    mult = mybir.AluOpType.mult
    add = mybir.AluOpType.add

    with TileContext(nc) as tc:
        with (
            tc.tile_pool(name="weights", bufs=1) as wpool,
            tc.tile_pool(name="state", bufs=1) as spool,
            tc.tile_pool(name="scratch", bufs=2) as scr,
            tc.tile_pool(name="psum", bufs=(2 if zero_bias else 1), space="PSUM") as pp,
            tc.tile_pool(name="psum_r", bufs=1, space="PSUM") as ppr,
            tc.tile_pool(name="outs", bufs=1) as opool,
        ):
            # ---- load weights/biases (once) ----
            w0sb = wpool.tile([128, KC, H], WDT, tag="w0")
            w1sb = wpool.tile([128, KC, H], WDT, tag="w1")
            wssb = wpool.tile([128, KC, H], WDT, tag="ws")
            nc.sync.dma_start(out=w0sb, in_=w0[:].rearrange("(k p) n -> p k n", p=128))
            nc.sync.dma_start(out=w1sb, in_=w1[:].rearrange("(k p) n -> p k n", p=128))
            nc.sync.dma_start(out=wssb, in_=ws[:].rearrange("(k p) n -> p k n", p=128))
            wr1sb = wpool.tile([128, KC, 128], F32, tag="wr1")
            nc.sync.dma_start(out=wr1sb, in_=wr1[:].rearrange("(k p) n -> p k n", p=128))
            wr2sb = wpool.tile([128, 6], F32, tag="wr2")
            nc.sync.dma_start(out=wr2sb, in_=wr2[:])

            b0sb = wpool.tile([128, KC], F32, tag="b0")
            b1sb = wpool.tile([128, KC], F32, tag="b1")
            bssb = wpool.tile([128, KC], F32, tag="bs")
            nc.sync.dma_start(out=b0sb, in_=b0[:].rearrange("(k p) -> p k", p=128))
            nc.sync.dma_start(out=b1sb, in_=b1[:].rearrange("(k p) -> p k", p=128))
            nc.sync.dma_start(out=bssb, in_=bs[:].rearrange("(k p) -> p k", p=128))
            br1sb = wpool.tile([128, 1], F32, tag="br1")
            nc.sync.dma_start(out=br1sb, in_=br1[:, None])
            br2sb = wpool.tile([6, 1], F32, tag="br2")
            nc.sync.dma_start(out=br2sb, in_=br2[:, None])

            if fold_s is None:
                # s broadcast to all 128 partitions: [128, BL]
                s_sb = wpool.tile([128, BL], F32, tag="s")
                s_src = svec[:]
                s_bcast = bass.AP(
                    tensor=s_src.tensor,
                    offset=s_src.offset,
                    ap=[[0, 128]] + list(s_src.ap),
                )
                nc.sync.dma_start(out=s_sb, in_=s_bcast)

                def s_ap():
                    t = s_sb[:, :]
                    return bass.AP(
                        tensor=t.tensor,
                        offset=t.offset,
                        ap=[t.ap[0], [0, KC], t.ap[1]],
                    )

            # ---- absorb weight-DMA sems into the PE vector clock ----
            # The S3_LW (weight-load) ISA struct allows only ONE sync wait;
            # a real matmul whose lhsT comes straight from a DMA plus an
            # rhs from another engine would need two.  These 1x1 dummy
            # matmuls each wait on one DMA queue, so every later PE
            # instruction sees those DMAs as already-observed.
            dummy_ps = ppr.tile([1, 1], F32, tag="dummy")

            def absorb(wtile):
                # 1x1 matmul whose only wait is wtile's DMA queue sem
                sl = wtile[:, 0, 0:1] if len(wtile.shape) == 3 else wtile[:, 0:1]
                nc.tensor.matmul(dummy_ps[:, :], lhsT=sl, rhs=sl,
                                 start=True, stop=True)

            absorb(w0sb)
            _pending_absorbs = [w1sb, wssb, wr1sb, wr2sb]

            # ---- state buffer: 10 slots of [128, KC, BL] ----
            zbuf = spool.tile([128, NSLOT, KC, BL], F32, tag="zbuf")
            nc.vector.memset(zbuf[:, 0], 0.0)

            # chunk groups: ZB pairs halves into [128,2,BL] ops (biases are
            # zero so one op can cover 2 H-chunks); otherwise per-chunk ops
            # with per-partition bias APs.
            groups = [(0, 2), (2, 2)] if zero_bias else [(0, 1), (1, 1), (2, 1), (3, 1)]

            def mlp_layer(in_t, w_t, b_t, func, out_t):
                """out_t[128,KC,BL] = func(chunked matmul(in_t) + bias).

                Per-group PSUM tiles; early groups' activations run on
                ACT/DVE while the PE streams later groups' matmuls."""
                is_relu = func is relu
                pss = [pp.tile([128, gw, BL], F32, name=f"ps{gi}", tag=f"ps{gi}")
                       for gi, (m0, gw) in enumerate(groups)]
                # k-split: contract over input chunks {0,1} for every output
                # chunk first, then {2,3} - the first half of the input is
                # activated earlier by the previous layer, so the PE can
                # start before the second half's DVE op lands.
                for kph in (range(0, KC // 2), range(KC // 2, KC)):
                    for gi, (m0, gw) in enumerate(groups):
                        for mm in range(gw):
                            m = m0 + mm
                            for k in kph:
                                nc.tensor.matmul(
                                    pss[gi][:, mm],
                                    lhsT=w_t[:, k, bass.ts(m, 128)],
                                    rhs=in_t[:, k, :],
                                    start=(k == 0),
                                    stop=(k == KC - 1),
                                    skip_group_check=True,
                                )
                for gi, (m0, gw) in enumerate(groups):
                    ps = pss[gi]
                    on_dve = is_relu and gi >= len(groups) // 2
                    if on_dve:
                        if zero_bias:
                            nc.vector.tensor_scalar(
                                out_t[:, m0:m0 + gw], ps[:], 0.0, None,
                                op0=mybir.AluOpType.max)
                        else:
                            nc.vector.tensor_scalar(
                                out_t[:, m0:m0 + gw], ps[:],
                                b_t[:, m0:m0 + 1], 0.0,
                                op0=add, op1=mybir.AluOpType.max)
                    else:
                        bias = 0.0 if zero_bias else b_t[:, m0:m0 + 1]
                        nc.scalar.activation(
                            out_t[:, m0:m0 + gw], ps[:], func,
                            bias=bias, scale=1.0)

            def mlp(in_t, utag):
                h1 = scr.tile([128, KC, BL], WDT, tag="h1")
                h2 = scr.tile([128, KC, BL], WDT, tag="h2")
                u = scr.tile([128, KC, BL], F32, tag=utag)
                mlp_layer(in_t, w0sb, b0sb, relu, h1)
                if _pending_absorbs:
                    absorb(_pending_absorbs.pop(0))   # w1 before L2's matmuls
                mlp_layer(h1, w1sb, b1sb, relu, h2)
                if _pending_absorbs:
                    absorb(_pending_absorbs.pop(0))   # ws before L3's matmuls
                mlp_layer(h2, wssb, bssb, tanh, u)
                while _pending_absorbs:
                    absorb(_pending_absorbs.pop(0))   # wr1/wr2 after eval 1
                return u

            def f32c(x):
                return float(np.float32(x))

            # ---- RK4 ----
            _znext_m = [None]
            for t in range(9):
                hf = f32c(hvals[t])
                half_h = f32c(np.float32(0.5) * np.float32(hf))
                h6 = f32c(np.float32(hf) / np.float32(6.0))
                z_t = zbuf[:, t]

                if wdt_lp:
                    if _znext_m[0] is not None:
                        ev1_in = _znext_m[0]
                    else:
                        zt_m = scr.tile([128, KC, BL], WDT, tag="ztm")
                        if zero_bias:
                            nc.vector.tensor_copy(zt_m[:, 0:2], z_t[:, 0:2])
                            nc.vector.tensor_copy(zt_m[:, 2:4], z_t[:, 2:4])
                        else:
                            nc.vector.tensor_copy(zt_m[:], z_t)
                        ev1_in = zt_m
                else:
                    ev1_in = z_t

                if fold_s is not None:
                    sv = np.float32(fold_s)
                    c1 = f32c(np.float32(half_h) * sv)   # 0.5*h*s
                    c2 = f32c(np.float32(hf) * sv)       # h*s
                    c3 = f32c(np.float32(h6) * sv)       # (h/6)*s

                    def halves(op):
                        for (m0, gw) in groups:
                            op(slice(m0, m0 + gw))

                    u1 = mlp(ev1_in, "u1")
                    zin1 = scr.tile([128, KC, BL], WDT, tag="zin")
                    halves(lambda h_: nc.vector.scalar_tensor_tensor(
                        out=zin1[:, h_], in0=u1[:, h_], scalar=c1,
                        in1=z_t[:, h_], op0=mult, op1=add))
                    u2 = mlp(zin1, "u2")
                    zin2 = scr.tile([128, KC, BL], WDT, tag="zin")
                    halves(lambda h_: nc.vector.scalar_tensor_tensor(
                        out=zin2[:, h_], in0=u2[:, h_], scalar=c1,
                        in1=z_t[:, h_], op0=mult, op1=add))
                    # acc = u1 + 2*u2 (+2*u3) accumulated mid-step, off the
                    # serial chain
                    acc = scr.tile([128, KC, BL], F32, tag="acc")
                    halves(lambda h_: nc.vector.scalar_tensor_tensor(
                        out=acc[:, h_], in0=u2[:, h_], scalar=2.0,
                        in1=u1[:, h_], op0=mult, op1=add))
                    u3 = mlp(zin2, "u3")
                    zin3 = scr.tile([128, KC, BL], WDT, tag="zin")
                    halves(lambda h_: nc.vector.scalar_tensor_tensor(
                        out=zin3[:, h_], in0=u3[:, h_], scalar=c2,
                        in1=z_t[:, h_], op0=mult, op1=add))
                    halves(lambda h_: nc.vector.scalar_tensor_tensor(
                        out=acc[:, h_], in0=u3[:, h_], scalar=2.0,
                        in1=acc[:, h_], op0=mult, op1=add))
                    u4 = mlp(zin3, "u4")
                    t4 = scr.tile([128, KC, BL], F32, tag="t4")
                    halves(lambda h_: nc.vector.tensor_add(
                        t4[:, h_], acc[:, h_], u4[:, h_]))
                    if wdt_lp:
                        # next step's matmul operand (bf16) first - it gates
                        # the PE; the f32 state write runs behind it
                        znext_m = scr.tile([128, KC, BL], WDT, tag="znm")
                        halves(lambda h_: nc.vector.scalar_tensor_tensor(
                            out=znext_m[:, h_], in0=t4[:, h_], scalar=c3,
                            in1=z_t[:, h_], op0=mult, op1=add))
                        _znext_m[0] = znext_m
                    halves(lambda h_: nc.vector.scalar_tensor_tensor(
                        out=zbuf[:, t + 1, h_], in0=t4[:, h_], scalar=c3,
                        in1=z_t[:, h_], op0=mult, op1=add))
                else:
                    u1 = mlp(ev1_in, "u1")
                    k1 = scr.tile([128, KC, BL], F32, tag="k1")
                    nc.vector.tensor_mul(k1[:], u1[:], s_ap())
                    zin1 = scr.tile([128, KC, BL], WDT, tag="zin")
                    nc.vector.scalar_tensor_tensor(
                        out=zin1[:], in0=k1[:], scalar=half_h, in1=z_t,
                        op0=mult, op1=add)
                    u2 = mlp(zin1, "u2")
                    k2 = scr.tile([128, KC, BL], F32, tag="k2")
                    nc.vector.tensor_mul(k2[:], u2[:], s_ap())
                    zin2 = scr.tile([128, KC, BL], WDT, tag="zin")
                    nc.vector.scalar_tensor_tensor(
                        out=zin2[:], in0=k2[:], scalar=half_h, in1=z_t,
                        op0=mult, op1=add)
                    u3 = mlp(zin2, "u3")
                    k3 = scr.tile([128, KC, BL], F32, tag="k3")
                    nc.vector.tensor_mul(k3[:], u3[:], s_ap())
                    zin3 = scr.tile([128, KC, BL], WDT, tag="zin")
                    nc.vector.scalar_tensor_tensor(
                        out=zin3[:], in0=k3[:], scalar=hf, in1=z_t,
                        op0=mult, op1=add)
                    u4 = mlp(zin3, "u4")
                    k4 = scr.tile([128, KC, BL], F32, tag="k4")
                    nc.vector.tensor_mul(k4[:], u4[:], s_ap())
                    acc = scr.tile([128, KC, BL], F32, tag="acc")
                    nc.vector.scalar_tensor_tensor(
                        out=acc[:], in0=k2[:], scalar=2.0, in1=k1[:],
                        op0=mult, op1=add)
                    nc.vector.scalar_tensor_tensor(
                        out=acc[:], in0=k3[:], scalar=2.0, in1=acc[:],
                        op0=mult, op1=add)
                    nc.vector.tensor_add(acc[:], acc[:], k4[:])
                    nc.vector.scalar_tensor_tensor(
                        out=zbuf[:, t + 1], in0=acc[:], scalar=h6, in1=z_t,
                        op0=mult, op1=add)

            # ---- regressor head ----
            ps_r = ppr.tile([128, NSLOT * BL], F32, tag="psr")
            for k in range(KC):
                nc.tensor.matmul(
                    ps_r[:, :],
                    lhsT=wr1sb[:, k, :],
                    rhs=zbuf[:, :, k, :],
                    start=(k == 0),
                    stop=(k == KC - 1),
                )
            # leaky_relu(x) = max(x, 0.1*x)  (exact for slope < 1)
            xr = opool.tile([128, NSLOT * BL], F32, tag="xr")
            nc.scalar.activation(
                xr[:, :], ps_r[:, :], ident,
                bias=br1sb[:, 0:1], scale=1.0,
            )
            xs = opool.tile([128, NSLOT * BL], F32, tag="xs")
            nc.vector.tensor_scalar_mul(xs[:, :], xr[:, :], 0.1)
            hr = opool.tile([128, NSLOT * BL], F32, tag="hr")
            nc.vector.tensor_max(hr[:, :], xr[:, :], xs[:, :])
            ps_p = ppr.tile([6, NSLOT * BL], F32, tag="psp")
            nc.tensor.matmul(ps_p[:, :], lhsT=wr2sb[:, :], rhs=hr[:, :],
                             start=True, stop=True)
            poses_sb = opool.tile([6, NSLOT * BL], F32, tag="poses")
            nc.scalar.activation(
                poses_sb[:, :], ps_p[:, :], ident,
                bias=br2sb[:, 0:1], scale=1.0,
            )
            nc.sync.dma_start(
                out=posesT[:],
                in_=poses_sb[:, :].rearrange("p (t b) -> p t b", t=NSLOT),
            )
            nc.sync.dma_start(out=hlastT[:], in_=zbuf[:, NSLOT - 1])

    nc.compile()
    return nc


_NC_CACHE = {}


def _get_nc(hsteps, wdt, fold_s, zero_bias):
    key = (tuple(float(x) for x in hsteps), wdt,
           None if fold_s is None else float(fold_s), bool(zero_bias))
    if key not in _NC_CACHE:
        _NC_CACHE[key] = _build_nc(list(key[0]), wdt, fold_s, zero_bias)
    return _NC_CACHE[key]


def prepare(fv, fi, ts, Wf0, bf0, Wf1, bf1, Wout, bout, Wr1, br1, Wr2, br2):
    """Build (nc, in_maps) for the current inputs/config."""
    fv = np.asarray(fv)  # unused mathematically (exact); see module docstring
    fi = np.asarray(fi)
    ts = np.asarray(ts, dtype=np.float32)
    Wf0 = np.ascontiguousarray(np.asarray(Wf0, dtype=np.float32))
    Wf1 = np.ascontiguousarray(np.asarray(Wf1, dtype=np.float32))
    Wr1 = np.ascontiguousarray(np.asarray(Wr1, dtype=np.float32))
    Wr2 = np.ascontiguousarray(np.asarray(Wr2, dtype=np.float32))
    Wsub = np.ascontiguousarray(np.asarray(Wout)[:, ::C].astype(np.float32))
    bsub = np.ascontiguousarray(np.asarray(bout)[::C].astype(np.float32))
    bf0 = np.asarray(bf0, dtype=np.float32)
    bf1 = np.asarray(bf1, dtype=np.float32)
    br1 = np.asarray(br1, dtype=np.float32)
    br2 = np.asarray(br2, dtype=np.float32)

    s_all = (ts[:, 2] - ts[:, 1]).astype(np.float32)          # [B]
    eval_t = np.linspace(0.1, 1.0, NSLOT, dtype=np.float32)
    hsteps = (eval_t[1:] - eval_t[:-1]).astype(np.float32)    # [9]

    wdt = os.environ.get("POSECDE_WDT", "bf16")
    fold_s = float(s_all[0]) if np.all(s_all == s_all[0]) else None
    zero_bias = (os.environ.get("POSECDE_ZB", "1") == "1"
                 and not bf0.any() and not bf1.any() and not bsub.any())

    nc = _get_nc(hsteps, wdt, fold_s, zero_bias)

    npdt = {"f32": np.float32, "bf16": ml_dtypes.bfloat16, "f16": np.float16}[wdt]
    w0_in = Wf0.astype(npdt)
    w1_in = Wf1.astype(npdt)
    ws_in = Wsub.astype(npdt)

    shared = dict(
        w0=w0_in, w1=w1_in, ws=ws_in, wr1=Wr1, wr2=Wr2,
        b0=bf0, b1=bf1, bs=bsub, br1=br1, br2=br2,
    )
    in_maps = []
    for i in range(N_CORES):
        m = dict(shared)
        m["svec"] = np.ascontiguousarray(s_all[i * BL : (i + 1) * BL])
        in_maps.append(m)
    return nc, in_maps


def assemble(results):
    poses = np.empty((B, NSLOT, 6), np.float32)
    h_last = np.empty((B, H), np.float32)
    for i in range(N_CORES):
        pT = results[i]["posesT"]               # [6, 10, BL]
        poses[i * BL : (i + 1) * BL] = pT.transpose(2, 1, 0)
        hT = results[i]["hlastT"]               # [128, KC, BL]
        h_last[i * BL : (i + 1) * BL] = hT.transpose(2, 1, 0).reshape(BL, H)
    return poses, h_last


def kernel(**inputs):
    nc, in_maps = prepare(**inputs)
    res = run_bass_kernel_spmd(nc, in_maps, list(range(N_CORES))).results
    return assemble(res)


# revision 20
# speedup vs baseline: 1.0263x; 1.0263x over previous
"""Trainium2 Bass kernel for nn_PoseCDE.

Mathematical structure exploited (exact, input-independent):
  The CDE integrates over t in [0.1, 1.0], which lies entirely inside the
  FIRST segment of the rectilinear control path (segment grid spacing is 1,
  bucketize gives idx=0 for all eval times).  Segment 0's derivative is the
  time-advance knot: (ts[b,2]-ts[b,1], 0, ..., 0).  Hence
      f(t, z) = s_b * g(z)[:, :, 0]
  and only column 0 of each C-group of Wout matters:  Wsub = Wout[:, ::C].
  The 512 x 262656 matmul collapses to 512 x 512 (verified exact vs the
  reference for arbitrary inputs).

Device computation per core (data-parallel over batch, 8 rows per core):
  36 sequential 3-layer MLP evals (RK4, 9 steps) + linear regressor head.
  All activations are kept TRANSPOSED ([H on partitions, batch on free]) so
  weights are the PE-stationary operand and no on-chip transposes are needed;
  biases become per-partition operands of scalar.activation.

Configs (env POSECDE_WDT = "bf16" | "f32", default bf16):
  bf16: MLP weights + matmul moving operands in bf16 (PSUM accum fp32,
        state/RK4 math fp32).  FWL halves weight-load time on the PE.
  When s (=ts[:,2]-ts[:,1]) is uniform across the batch it is folded into
  the RK4 step scalars (fewer DVE ops, shorter serial chain).
"""

import os
import numpy as np
import ml_dtypes

import concourse.bass as bass
import concourse.bacc as bacc
import concourse.mybir as mybir
from concourse.tile import TileContext
from concourse.bass_utils import run_bass_kernel_spmd

N_CORES = 8
B = 64
BL = B // N_CORES          # batch rows per core
H = 512
C = H + 1
KC = H // 128              # contraction chunks (4)
NSLOT = 10                 # z0 + 9 RK4 states
F32 = mybir.dt.float32
BF16 = mybir.dt.bfloat16
F16 = mybir.dt.float16


def _build_nc(hvals, wdt, fold_s, zero_bias):
    """wdt: "f32"|"bf16"|"f16"; fold_s: None or uniform-s float;
    zero_bias: True specializes the MLP biases to 0 (merged-chunk ACT)."""
    nc = bacc.Bacc(None)
    WDT = {"f32": F32, "bf16": BF16, "f16": F16}[wdt]
    wdt_lp = wdt != "f32"

    w0 = nc.declare_dram_parameter("w0", [H, H], WDT, isOutput=False)
    w1 = nc.declare_dram_parameter("w1", [H, H], WDT, isOutput=False)
    ws = nc.declare_dram_parameter("ws", [H, H], WDT, isOutput=False)
    wr1 = nc.declare_dram_parameter("wr1", [H, 128], WDT, isOutput=False)
    wr2 = nc.declare_dram_parameter("wr2", [128, 6], F32, isOutput=False)
    b0 = nc.declare_dram_parameter("b0", [H], F32, isOutput=False)
    b1 = nc.declare_dram_parameter("b1", [H], F32, isOutput=False)
    bs = nc.declare_dram_parameter("bs", [H], F32, isOutput=False)
    br1 = nc.declare_dram_parameter("br1", [128], F32, isOutput=False)
    br2 = nc.declare_dram_parameter("br2", [6], F32, isOutput=False)
    svec = nc.declare_dram_parameter("svec", [BL], F32, isOutput=False)

    posesT = nc.declare_dram_parameter("posesT", [6, NSLOT, BL], F32, isOutput=True)
    hlastT = nc.declare_dram_parameter("hlastT", [128, KC, BL], F32, isOutput=True)

    relu = mybir.ActivationFunctionType.Relu
    tanh = mybir.ActivationFunctionType.Tanh
    ident = mybir.ActivationFunctionType.Identity
    mult = mybir.AluOpType.mult
    add = mybir.AluOpType.add

    with TileContext(nc) as tc:
        with (
            tc.tile_pool(name="weights", bufs=1) as wpool,
            tc.tile_pool(name="state", bufs=1) as spool,
            tc.tile_pool(name="scratch", bufs=2) as scr,
            tc.tile_pool(name="psum", bufs=(2 if zero_bias else 1), space="PSUM") as pp,
            tc.tile_pool(name="psum_r", bufs=1, space="PSUM") as ppr,
            tc.tile_pool(name="outs", bufs=1) as opool,
        ):
            # ---- load weights/biases (once) ----
            w0sb = wpool.tile([128, KC, H], WDT, tag="w0")
            w1sb = wpool.tile([128, KC, H], WDT, tag="w1")
            wssb = wpool.tile([128, KC, H], WDT, tag="ws")
            nc.sync.dma_start(out=w0sb, in_=w0[:].rearrange("(k p) n -> p k n", p=128))
            nc.sync.dma_start(out=w1sb, in_=w1[:].rearrange("(k p) n -> p k n", p=128))
            nc.sync.dma_start(out=wssb, in_=ws[:].rearrange("(k p) n -> p k n", p=128))
            wr1sb = wpool.tile([128, KC, 128], WDT, tag="wr1")
            nc.sync.dma_start(out=wr1sb, in_=wr1[:].rearrange("(k p) n -> p k n", p=128))
            wr2sb = wpool.tile([128, 6], F32, tag="wr2")
            nc.sync.dma_start(out=wr2sb, in_=wr2[:])

            b0sb = wpool.tile([128, KC], F32, tag="b0")
            b1sb = wpool.tile([128, KC], F32, tag="b1")
            bssb = wpool.tile([128, KC], F32, tag="bs")
            nc.sync.dma_start(out=b0sb, in_=b0[:].rearrange("(k p) -> p k", p=128))
            nc.sync.dma_start(out=b1sb, in_=b1[:].rearrange("(k p) -> p k", p=128))
            nc.sync.dma_start(out=bssb, in_=bs[:].rearrange("(k p) -> p k", p=128))
            br1sb = wpool.tile([128, 1], F32, tag="br1")
            nc.sync.dma_start(out=br1sb, in_=br1[:, None])
            br2sb = wpool.tile([6, 1], F32, tag="br2")
            nc.sync.dma_start(out=br2sb, in_=br2[:, None])

            if fold_s is None:
                # s broadcast to all 128 partitions: [128, BL]
                s_sb = wpool.tile([128, BL], F32, tag="s")
                s_src = svec[:]
                s_bcast = bass.AP(
                    tensor=s_src.tensor,
                    offset=s_src.offset,
                    ap=[[0, 128]] + list(s_src.ap),
                )
                nc.sync.dma_start(out=s_sb, in_=s_bcast)

                def s_ap():
                    t = s_sb[:, :]
                    return bass.AP(
                        tensor=t.tensor,
                        offset=t.offset,
                        ap=[t.ap[0], [0, KC], t.ap[1]],
                    )

            # ---- absorb weight-DMA sems into the PE vector clock ----
            # The S3_LW (weight-load) ISA struct allows only ONE sync wait;
            # a real matmul whose lhsT comes straight from a DMA plus an
            # rhs from another engine would need two.  These 1x1 dummy
            # matmuls each wait on one DMA queue, so every later PE
            # instruction sees those DMAs as already-observed.
            dummy_ps = ppr.tile([1, 1], F32, tag="dummy")

            def absorb(wtile):
                # 1x1 matmul whose only wait is wtile's DMA queue sem
                sl = wtile[:, 0, 0:1] if len(wtile.shape) == 3 else wtile[:, 0:1]
                nc.tensor.matmul(dummy_ps[:, :], lhsT=sl, rhs=sl,
                                 start=True, stop=True)

            absorb(w0sb)
            _pending_absorbs = [w1sb, wssb, wr1sb, wr2sb]

            # ---- state buffers: 10 slots of [128, KC, BL] ----
            # zbuf: fp32 state (exact, feeds h_last output + fp32 matmuls)
            # zbuf_m: low-precision copy feeding the PE when wdt_lp
            zbuf = spool.tile([128, NSLOT, KC, BL], F32, tag="zbuf")
            nc.vector.memset(zbuf[:, 0], 0.0)
            if wdt_lp:
                zbuf_m = spool.tile([128, NSLOT, KC, BL], WDT, tag="zbufm")
                nc.vector.memset(zbuf_m[:, 0], 0.0)

            # chunk groups: ZB pairs halves into [128,2,BL] ops (biases are
            # zero so one op can cover 2 H-chunks); otherwise per-chunk ops
            # with per-partition bias APs.
            groups = [(0, 2), (2, 2)] if zero_bias else [(0, 1), (1, 1), (2, 1), (3, 1)]

            def mlp_layer(in_t, w_t, b_t, func, out_t):
                """out_t[128,KC,BL] = func(chunked matmul(in_t) + bias).

                Per-group PSUM tiles; early groups' activations run on
                ACT/DVE while the PE streams later groups' matmuls."""
                is_relu = func is relu
                pss = [pp.tile([128, gw, BL], F32, name=f"ps{gi}", tag=f"ps{gi}")
                       for gi, (m0, gw) in enumerate(groups)]
                # k-split: contract over input chunks {0,1} for every output
                # chunk first, then {2,3} - the first half of the input is
                # activated earlier by the previous layer, so the PE can
                # start before the second half's DVE op lands.
                for kph in (range(0, KC // 2), range(KC // 2, KC)):
                    for gi, (m0, gw) in enumerate(groups):
                        for mm in range(gw):
                            m = m0 + mm
                            for k in kph:
                                nc.tensor.matmul(
                                    pss[gi][:, mm],
                                    lhsT=w_t[:, k, bass.ts(m, 128)],
                                    rhs=in_t[:, k, :],
                                    start=(k == 0),
                                    stop=(k == KC - 1),
                                    skip_group_check=True,
                                )
                for gi, (m0, gw) in enumerate(groups):
                    ps = pss[gi]
                    on_dve = is_relu and gi < len(groups) // 2
                    if on_dve:
                        if zero_bias:
                            nc.vector.tensor_scalar(
                                out_t[:, m0:m0 + gw], ps[:], 0.0, None,
                                op0=mybir.AluOpType.max)
                        else:
                            nc.vector.tensor_scalar(
                                out_t[:, m0:m0 + gw], ps[:],
                                b_t[:, m0:m0 + 1], 0.0,
                                op0=add, op1=mybir.AluOpType.max)
                    else:
                        bias = 0.0 if zero_bias else b_t[:, m0:m0 + 1]
                        nc.scalar.activation(
                            out_t[:, m0:m0 + gw], ps[:], func,
                            bias=bias, scale=1.0)

            def mlp(in_t, utag):
                h1 = scr.tile([128, KC, BL], WDT, tag="h1")
                h2 = scr.tile([128, KC, BL], WDT, tag="h2")
                u = scr.tile([128, KC, BL], F32, tag=utag)
                mlp_layer(in_t, w0sb, b0sb, relu, h1)
                if _pending_absorbs:
                    absorb(_pending_absorbs.pop(0))   # w1 before L2's matmuls
                mlp_layer(h1, w1sb, b1sb, relu, h2)
                if _pending_absorbs:
                    absorb(_pending_absorbs.pop(0))   # ws before L3's matmuls
                mlp_layer(h2, wssb, bssb, tanh, u)
                while _pending_absorbs:
                    absorb(_pending_absorbs.pop(0))   # wr1/wr2 after eval 1
                return u

            def f32c(x):
                return float(np.float32(x))

            # ---- RK4 ----
            for t in range(9):
                hf = f32c(hvals[t])
                half_h = f32c(np.float32(0.5) * np.float32(hf))
                h6 = f32c(np.float32(hf) / np.float32(6.0))
                z_t = zbuf[:, t]

                if wdt_lp:
                    if fold_s is None and t > 0:
                        # general path keeps only fp32 state; cast per step
                        zt_m = scr.tile([128, KC, BL], WDT, tag="ztm")
                        nc.vector.tensor_copy(zt_m[:], z_t)
                        ev1_in = zt_m
                    else:
                        ev1_in = zbuf_m[:, t]
                else:
                    ev1_in = z_t

                if fold_s is not None:
                    sv = np.float32(fold_s)
                    c1 = f32c(np.float32(half_h) * sv)   # 0.5*h*s
                    c2 = f32c(np.float32(hf) * sv)       # h*s
                    c3 = f32c(np.float32(h6) * sv)       # (h/6)*s

                    def halves(op):
                        for (m0, gw) in groups:
                            op(slice(m0, m0 + gw))

                    u1 = mlp(ev1_in, "u1")
                    zin1 = scr.tile([128, KC, BL], WDT, tag="zin")
                    halves(lambda h_: nc.vector.scalar_tensor_tensor(
                        out=zin1[:, h_], in0=u1[:, h_], scalar=c1,
                        in1=z_t[:, h_], op0=mult, op1=add))
                    u2 = mlp(zin1, "u2")
                    zin2 = scr.tile([128, KC, BL], WDT, tag="zin")
                    halves(lambda h_: nc.vector.scalar_tensor_tensor(
                        out=zin2[:, h_], in0=u2[:, h_], scalar=c1,
                        in1=z_t[:, h_], op0=mult, op1=add))
                    # acc = u1 + 2*u2 (+2*u3) accumulated mid-step, off the
                    # serial chain
                    acc = scr.tile([128, KC, BL], F32, tag="acc")
                    halves(lambda h_: nc.vector.scalar_tensor_tensor(
                        out=acc[:, h_], in0=u2[:, h_], scalar=2.0,
                        in1=u1[:, h_], op0=mult, op1=add))
                    u3 = mlp(zin2, "u3")
                    zin3 = scr.tile([128, KC, BL], WDT, tag="zin")
                    halves(lambda h_: nc.vector.scalar_tensor_tensor(
                        out=zin3[:, h_], in0=u3[:, h_], scalar=c2,
                        in1=z_t[:, h_], op0=mult, op1=add))
                    halves(lambda h_: nc.vector.scalar_tensor_tensor(
                        out=acc[:, h_], in0=u3[:, h_], scalar=2.0,
                        in1=acc[:, h_], op0=mult, op1=add))
                    u4 = mlp(zin3, "u4")
                    t4 = scr.tile([128, KC, BL], F32, tag="t4")
                    for (m0, gw) in groups:
                        h_ = slice(m0, m0 + gw)
                        nc.vector.tensor_add(t4[:, h_], acc[:, h_], u4[:, h_])
                        if wdt_lp:
                            # low-precision state first - it gates the PE
                            nc.vector.scalar_tensor_tensor(
                                out=zbuf_m[:, t + 1, h_], in0=t4[:, h_],
                                scalar=c3, in1=z_t[:, h_], op0=mult, op1=add)
                    halves(lambda h_: nc.vector.scalar_tensor_tensor(
                        out=zbuf[:, t + 1, h_], in0=t4[:, h_], scalar=c3,
                        in1=z_t[:, h_], op0=mult, op1=add))
                else:
                    u1 = mlp(ev1_in, "u1")
                    k1 = scr.tile([128, KC, BL], F32, tag="k1")
                    nc.vector.tensor_mul(k1[:], u1[:], s_ap())
                    zin1 = scr.tile([128, KC, BL], WDT, tag="zin")
                    nc.vector.scalar_tensor_tensor(
                        out=zin1[:], in0=k1[:], scalar=half_h, in1=z_t,
                        op0=mult, op1=add)
                    u2 = mlp(zin1, "u2")
                    k2 = scr.tile([128, KC, BL], F32, tag="k2")
                    nc.vector.tensor_mul(k2[:], u2[:], s_ap())
                    zin2 = scr.tile([128, KC, BL], WDT, tag="zin")
                    nc.vector.scalar_tensor_tensor(
                        out=zin2[:], in0=k2[:], scalar=half_h, in1=z_t,
                        op0=mult, op1=add)
                    u3 = mlp(zin2, "u3")
                    k3 = scr.tile([128, KC, BL], F32, tag="k3")
                    nc.vector.tensor_mul(k3[:], u3[:], s_ap())
                    zin3 = scr.tile([128, KC, BL], WDT, tag="zin")
                    nc.vector.scalar_tensor_tensor(
                        out=zin3[:], in0=k3[:], scalar=hf, in1=z_t,
                        op0=mult, op1=add)
                    u4 = mlp(zin3, "u4")
                    k4 = scr.tile([128, KC, BL], F32, tag="k4")
                    nc.vector.tensor_mul(k4[:], u4[:], s_ap())
                    acc = scr.tile([128, KC, BL], F32, tag="acc")
                    nc.vector.scalar_tensor_tensor(
                        out=acc[:], in0=k2[:], scalar=2.0, in1=k1[:],
                        op0=mult, op1=add)
                    nc.vector.scalar_tensor_tensor(
                        out=acc[:], in0=k3[:], scalar=2.0, in1=acc[:],
                        op0=mult, op1=add)
                    nc.vector.tensor_add(acc[:], acc[:], k4[:])
                    nc.vector.scalar_tensor_tensor(
                        out=zbuf[:, t + 1], in0=acc[:], scalar=h6, in1=z_t,
                        op0=mult, op1=add)

            # ---- regressor head ----
            if wdt_lp and fold_s is not None:
                zreg = zbuf_m
            elif wdt_lp:
                # general low-precision path: cast the full state buffer once
                zreg = spool.tile([128, NSLOT, KC, BL], WDT, tag="zregm")
                nc.vector.tensor_copy(zreg[:], zbuf[:])
                zreg = zreg  # noqa
            else:
                zreg = zbuf
            ps_r = ppr.tile([128, NSLOT * BL], F32, tag="psr")
            for k in range(KC):
                nc.tensor.matmul(
                    ps_r[:, :],
                    lhsT=wr1sb[:, k, :],
                    rhs=zreg[:, :, k, :],
                    start=(k == 0),
                    stop=(k == KC - 1),
                )
            # leaky_relu(x) = max(x, 0.1*x)  (exact for slope < 1)
            xr = opool.tile([128, NSLOT * BL], F32, tag="xr")
            nc.scalar.activation(
                xr[:, :], ps_r[:, :], ident,
                bias=br1sb[:, 0:1], scale=1.0,
            )
            xs = opool.tile([128, NSLOT * BL], F32, tag="xs")
            nc.vector.tensor_scalar_mul(xs[:, :], xr[:, :], 0.1)
            hr = opool.tile([128, NSLOT * BL], F32, tag="hr")
            nc.vector.tensor_max(hr[:, :], xr[:, :], xs[:, :])
            ps_p = ppr.tile([6, NSLOT * BL], F32, tag="psp")
            nc.tensor.matmul(ps_p[:, :], lhsT=wr2sb[:, :], rhs=hr[:, :],
                             start=True, stop=True)
            poses_sb = opool.tile([6, NSLOT * BL], F32, tag="poses")
            nc.scalar.activation(
                poses_sb[:, :], ps_p[:, :], ident,
                bias=br2sb[:, 0:1], scale=1.0,
            )
            nc.sync.dma_start(
                out=posesT[:],
                in_=poses_sb[:, :].rearrange("p (t b) -> p t b", t=NSLOT),
            )
            nc.sync.dma_start(out=hlastT[:], in_=zbuf[:, NSLOT - 1])

    nc.compile()
    return nc


_NC_CACHE = {}


def _get_nc(hsteps, wdt, fold_s, zero_bias):
    key = (tuple(float(x) for x in hsteps), wdt,
           None if fold_s is None else float(fold_s), bool(zero_bias))
    if key not in _NC_CACHE:
        _NC_CACHE[key] = _build_nc(list(key[0]), wdt, fold_s, zero_bias)
    return _NC_CACHE[key]


def prepare(fv, fi, ts, Wf0, bf0, Wf1, bf1, Wout, bout, Wr1, br1, Wr2, br2):
    """Build (nc, in_maps) for the current inputs/config."""
    fv = np.asarray(fv)  # unused mathematically (exact); see module docstring
    fi = np.asarray(fi)
    ts = np.asarray(ts, dtype=np.float32)
    Wf0 = np.ascontiguousarray(np.asarray(Wf0, dtype=np.float32))
    Wf1 = np.ascontiguousarray(np.asarray(Wf1, dtype=np.float32))
    Wr1 = np.ascontiguousarray(np.asarray(Wr1, dtype=np.float32))
    Wr2 = np.ascontiguousarray(np.asarray(Wr2, dtype=np.float32))
    Wsub = np.ascontiguousarray(np.asarray(Wout)[:, ::C].astype(np.float32))
    bsub = np.ascontiguousarray(np.asarray(bout)[::C].astype(np.float32))
    bf0 = np.asarray(bf0, dtype=np.float32)
    bf1 = np.asarray(bf1, dtype=np.float32)
    br1 = np.asarray(br1, dtype=np.float32)
    br2 = np.asarray(br2, dtype=np.float32)

    s_all = (ts[:, 2] - ts[:, 1]).astype(np.float32)          # [B]
    eval_t = np.linspace(0.1, 1.0, NSLOT, dtype=np.float32)
    hsteps = (eval_t[1:] - eval_t[:-1]).astype(np.float32)    # [9]

    wdt = os.environ.get("POSECDE_WDT", "bf16")
    fold_s = float(s_all[0]) if np.all(s_all == s_all[0]) else None
    zero_bias = (os.environ.get("POSECDE_ZB", "1") == "1"
                 and not bf0.any() and not bf1.any() and not bsub.any())

    nc = _get_nc(hsteps, wdt, fold_s, zero_bias)

    npdt = {"f32": np.float32, "bf16": ml_dtypes.bfloat16, "f16": np.float16}[wdt]
    w0_in = Wf0.astype(npdt)
    w1_in = Wf1.astype(npdt)
    ws_in = Wsub.astype(npdt)
    wr1_in = Wr1.astype(npdt)

    shared = dict(
        w0=w0_in, w1=w1_in, ws=ws_in, wr1=wr1_in, wr2=Wr2,
        b0=bf0, b1=bf1, bs=bsub, br1=br1, br2=br2,
    )
    in_maps = []
    for i in range(N_CORES):
        m = dict(shared)
        m["svec"] = np.ascontiguousarray(s_all[i * BL : (i + 1) * BL])
        in_maps.append(m)
    return nc, in_maps


def assemble(results):
    poses = np.empty((B, NSLOT, 6), np.float32)
    h_last = np.empty((B, H), np.float32)
    for i in range(N_CORES):
        pT = results[i]["posesT"]               # [6, 10, BL]
        poses[i * BL : (i + 1) * BL] = pT.transpose(2, 1, 0)
        hT = results[i]["hlastT"]               # [128, KC, BL]
        h_last[i * BL : (i + 1) * BL] = hT.transpose(2, 1, 0).reshape(BL, H)
    return poses, h_last


def kernel(**inputs):
    nc, in_maps = prepare(**inputs)
    res = run_bass_kernel_spmd(nc, in_maps, list(range(N_CORES))).results
    return assemble(res)
